# revision 64
# baseline (speedup 1.0000x reference)
"""Trainium2 Bass kernel for causal self-attention (GQA + RoPE).

Problem: B=2, T=2048, n_embd=4096, HQ=32 q-heads, HKV=8 kv-heads, HD=128.
  q = rope(x @ wq), k = rope(x @ wk), v = x @ wv
  y = causal_softmax(q k^T / sqrt(HD)) v @ wproj

Sharding (8 cores): core = (b, g), b in {0,1} batch, g in {0..3} head-group.
Each core handles 8 q-heads / 2 kv-heads of one batch sample:
  - wq/wk/wv column-sharded, wproj row-sharded (tensor parallel over heads)
  - final reduce (sum of 4 partial y per batch) done on host in fp32.

Per-core device program. The four big GEMMs (QKV projections and the
output projection) run as error-compensated fp8 DoubleRow matmuls:
operands are split host/device-side into e4m3 hi+lo pairs and the three
significant cross products (hi*hi, hi*lo, lo*hi) are evaluated with
paired-contraction DoubleRow instructions, giving 0.75x the bf16 PE time
at better-than-bf16 accuracy. Weights are pre-scaled by 64 so their
values sit in e4m3's normal range; the 1/64 is folded into the RoPE
tables (q/k path), the rowsum ones-column (v path, set to 64 so the
softmax normalization cancels the v scale), and the host-side output
reduce. Attention itself (scores, exp, P@V) stays bf16.

  A) projections: Q^T,K^T per head [d=128 part, t free] with fused RoPE;
     V^T then PE-transposed into V[tok, dv] with a 64-valued column
     appended (rowsum*64 rides in col 128).
  B) attention per (head, 512-token q-chunk): S^T = K-block^T-matmul(Q^T),
     ACT exp -> P^T (bf16), 0/1 causal mask multiplied into P's diag
     blocks on GPSIMD (off the score->exp chain, hidden by the vmms
     pipeline slack), then out[tq,129] += P^T-block.T @ [V64|64],
     normalize (gives unscaled a), PE-transpose -> a^T, DVE-split into
     fp8 hi/lo planes.
  C) y64^T = wproj64_s^T-blocks @ a^T (fp8 3-term), fp32 eviction, DMA.
"""
import sys

if "/opt/trn_rl_repo" not in sys.path:
    sys.path.insert(0, "/opt/trn_rl_repo")

import math
import numpy as np
import ml_dtypes

B, T, N_EMBD = 2, 2048, 4096
HQ, HKV = 32, 8
HD = 128
N_CORES = 8
TPG = 4                      # tensor-parallel groups per batch
HQL, HKVL = HQ // TPG, HKV // TPG   # 8 q-heads, 2 kv-heads per core
SCALE = 1.0 / math.sqrt(HD)
BASE_FREQ = 10000.0
NEG = -1e30
WS = 64.0                    # weight pre-scale (fp8 subnormal avoidance)

bf16 = ml_dtypes.bfloat16
e4m3 = ml_dtypes.float8_e4m3


def build_nc(T=T, KE=N_EMBD, HQL=HQL, HKVL=HKVL, EOUT=N_EMBD, scale=SCALE):
    """Build the per-core Bass program. All shapes hardcoded at trace time."""
    import concourse.tile as tile
    from concourse import bacc, mybir

    f32 = mybir.dt.float32
    b16 = mybir.dt.bfloat16
    f8 = mybir.dt.float8e4
    Exp = mybir.ActivationFunctionType.Exp
    mult = mybir.AluOpType.mult
    add = mybir.AluOpType.add
    sub = mybir.AluOpType.subtract
    DR = mybir.MatmulPerfMode.DoubleRow

    KT = KE // 128          # contraction tiles for projections
    NKT = T // 128          # token tiles
    NCH = T // 512          # token chunks
    REP = HQL // HKVL

    nc = bacc.Bacc("TRN2", target_bir_lowering=False)

    # x^T hi/lo fp8 planes interleaved per k-tile
    xs_d = nc.dram_tensor("xs", [128, KT, 2, T], f8, kind="ExternalInput")
    wq8_d = nc.dram_tensor("wq8", [128, HQL, KT, 128], f8, kind="ExternalInput")
    wqr_d = nc.dram_tensor("wqr", [128, HQL, KT, 128], f8, kind="ExternalInput")
    wk8_d = nc.dram_tensor("wk8", [128, HKVL, KT, 128], f8, kind="ExternalInput")
    wkr_d = nc.dram_tensor("wkr", [128, HKVL, KT, 128], f8, kind="ExternalInput")
    wv8_d = nc.dram_tensor("wv8", [128, HKVL, KT, 128], f8, kind="ExternalInput")
    wvr_d = nc.dram_tensor("wvr", [128, HKVL, KT, 128], f8, kind="ExternalInput")
    wp8_d = nc.dram_tensor("wp8", [128, HQL, EOUT], f8, kind="ExternalInput")
    wpr_d = nc.dram_tensor("wpr", [128, HQL, EOUT], f8, kind="ExternalInput")
    cos_d = nc.dram_tensor("cos", [128, T], b16, kind="ExternalInput")
    sin_d = nc.dram_tensor("rsin", [128, T // 2], b16, kind="ExternalInput")
    tri_d = nc.dram_tensor("tri", [128, 128], b16, kind="ExternalInput")
    id_d = nc.dram_tensor("ident", [128, 128], b16, kind="ExternalInput")
    yt_d = nc.dram_tensor("yt", [EOUT, T], b16, kind="ExternalOutput")

    NG = KT // 2            # DoubleRow pair-groups over projection k-tiles
    P0_OFF = [0, 512, 896, 1152]   # compact offsets of P^T(c0,h0) tiles

    with tile.TileContext(nc) as tc:
        with tc.tile_pool(name="glob", bufs=1) as glob:
            cos_sb = glob.tile([128, T], b16)
            sin_sb = glob.tile([128, T // 2], b16)
            # P^T(c=0, h=0), prewarmed at the end of phase A; tile t
            # (width 512-128t) stored compactly at P0_OFF[t].
            p0 = glob.tile([128, 1280], b16)
            tri_sb = glob.tile([128, 128], b16)
            id_sb = glob.tile([128, 128], b16)

            qT = glob.tile([128, HQL, T], b16)       # rope(q)^T per head
            kT = glob.tile([128, HKVL, T], b16)      # rope(k)^T per head
            vON = glob.tile([128, HKVL, NKT, 129], b16)  # [tok, 64*dv | 64]
            nc.vector.memset(vON[:, :, :, 128:129], WS)

            # ---------------- Phase A: projections -------------------------
            with tc.tile_pool(name="xs", bufs=1) as xsp, \
                 tc.tile_pool(name="wld", bufs=4) as wld, \
                 tc.tile_pool(name="rtmp", bufs=1) as rtmp, \
                 tc.tile_pool(name="vtmp", bufs=1) as vtmp, \
                 tc.tile_pool(name="psA", bufs=7, space="PSUM") as psA, \
                 tc.tile_pool(name="psT", bufs=1, space="PSUM") as psT:

                def load_w(hi_d_, lo_d_, m):
                    wh = wld.tile([128, KT, 128], f8, tag="w", name="wh")
                    nc.sync.dma_start(out=wh[:], in_=hi_d_[:, m, :, :])
                    wl = wld.tile([128, KT, 128], f8, tag="w", name="wl")
                    nc.sync.dma_start(out=wl[:], in_=lo_d_[:, m, :, :])
                    return wh, wl

                # First two weight slab pairs interleave with the leading xs
                # tiles so PE's first matmuls start as early as possible.
                xs_sb = xsp.tile([128, KT, 2, T], f8)
                wtiles = [wld.tile([128, KT, 128], f8, tag="w",
                                   name=f"w{i}") for i in range(4)]
                slabs = {-1: (wq8_d, 0, 0), 0: (wqr_d, 0, 1),
                         1: (wq8_d, 1, 2), 2: (wqr_d, 1, 3)}
                KH = KT // 2
                for a in range(-1, KT):
                    if a in slabs:
                        # k-halves: matmuls on low k unblock after half the
                        # slab's bytes land.
                        w_d_, m, i = slabs[a]
                        nc.sync.dma_start(out=wtiles[i][:, 0:KH, :],
                                          in_=w_d_[:, m, 0:KH, :])
                        nc.sync.dma_start(out=wtiles[i][:, KH:KT, :],
                                          in_=w_d_[:, m, KH:KT, :])
                    if a < 0:
                        continue
                    if a < 6:
                        # column-halves for the leading tiles: chunk-0/1
                        # units start after half a tile's bytes.
                        nc.sync.dma_start(out=xs_sb[:, a, :, 0:T // 2],
                                          in_=xs_d[:, a, :, 0:T // 2])
                        nc.sync.dma_start(out=xs_sb[:, a, :, T // 2:T],
                                          in_=xs_d[:, a, :, T // 2:T])
                    else:
                        nc.sync.dma_start(out=xs_sb[:, a, :, :],
                                          in_=xs_d[:, a, :, :])
                    if a == KT - 1:  # tables last: ropes need them only late
                        nc.sync.dma_start(out=cos_sb[:], in_=cos_d[:])
                        nc.sync.dma_start(out=sin_sb[:], in_=sin_d[:])
                        nc.sync.dma_start(out=tri_sb[:], in_=tri_d[:])
                        nc.sync.dma_start(out=id_sb[:], in_=id_d[:])
                w_first = (wtiles[0], wtiles[1])
                w_m1 = (wtiles[2], wtiles[3])

                def mm3_hi(ps, wpair, c, g, first):
                    """hi-weight products of pair-group g into ps:
                    (w8[k],x8[k])+(w8[k],xr8[k]) for k=2g,2g+1."""
                    wh, _ = wpair
                    cs = slice(512 * c, 512 * (c + 1))
                    for k in (2 * g, 2 * g + 1):
                        nc.tensor.matmul(
                            ps[:],
                            lhsT=wh[:, k, :].unsqueeze(1)
                                .broadcast_to([128, 2, 128]),
                            rhs=xs_sb[:, k, :, cs],
                            start=(first and k == 2 * g), stop=False,
                            perf_mode=DR)

                def mm3_lo(ps, wpair, c, g, last):
                    """lo-weight products of pair-group g into ps:
                    (wr[2g],x8[2g])+(wr[2g+1],x8[2g+1])."""
                    _, wl = wpair
                    k0 = 2 * g
                    cs = slice(512 * c, 512 * (c + 1))
                    nc.tensor.matmul(
                        ps[:],
                        lhsT=wl[:, k0:k0 + 2, :],
                        rhs=xs_sb[:, k0:k0 + 2, 0, cs],
                        start=False, stop=last, perf_mode=DR)

                def mm3(ps, wpair, c, g, last):
                    mm3_hi(ps, wpair, c, g, g == 0)
                    mm3_lo(ps, wpair, c, g, last)

                def rope_evict(ps, dst, c):
                    # dst = ps * cos + rot64(ps) * sin  (bf16 out);
                    # rot[0:64] = -ps[64:128], rot[64:128] = ps[0:64]
                    # tables carry the 1/WS weight-scale compensation.
                    # sin is packed [128, T/2]: chunks 0/1 in rows 0:64,
                    # chunks 2/3 in rows 64:128.
                    cs = slice(512 * c, 512 * (c + 1))
                    sr = slice(0, 64) if c < 2 else slice(64, 128)
                    scs = slice(512 * (c % 2), 512 * (c % 2 + 1))
                    sin_c = sin_sb[sr, scs]
                    t1 = rtmp.tile([128, 512], f32, tag="t1")
                    nc.vector.scalar_tensor_tensor(
                        t1[0:64, :], ps[64:128, :], -1.0, sin_c,
                        op0=mult, op1=mult)
                    nc.vector.tensor_tensor(t1[64:128, :], ps[0:64, :],
                                            sin_c, mult)
                    t2 = rtmp.tile([128, 512], f32, tag="t2")
                    nc.vector.tensor_tensor(t2[:], ps[:], cos_sb[:, cs], mult)
                    nc.vector.tensor_tensor(dst, t2[:], t1[:], add)

                # Startup ramp: q-heads 0+1 run pair-group-outer, interleaved
                # over 7 live psums (m0 all 4 chunks + m1 chunks 0-2) so PE
                # issues work per freshly-landed xs tile and tracks the DMA.
                units = [(0, c) for c in range(NCH)] + \
                        [(1, c) for c in range(NCH)]
                wfns = {0: w_first, 1: w_m1}
                pss = {u: psA.tile([128, 512], f32, tag="pj",
                                   name=f"pj{u[0]}_{u[1]}")
                       for u in units[:-1]}
                pss[units[-1]] = psT.tile([128, 512], f32, tag="tr",
                                          name="pj8")
                # hi-products run LAG groups ahead of lo-products so the hi
                # slabs' last read lands early, freeing their buffers for the
                # next head's prefetch while the lo tail still runs.
                LAG = 2
                for g in range(NG + LAG):
                    for (m, c) in units:
                        if g < NG:
                            mm3_hi(pss[(m, c)], wfns[m], c, g, g == 0)
                        if g >= LAG:
                            mm3_lo(pss[(m, c)], wfns[m], c, g - LAG,
                                   g - LAG == NG - 1)
                for (m, c) in units:
                    rope_evict(pss[(m, c)], qT[:, m, 512 * c:512 * (c + 1)], c)

                # remaining projections (q-heads 2-7, k-heads, v-heads),
                # weight slabs prefetched one head ahead so their DMA hides
                # under the previous head's matmuls.
                rest = [(qT, wq8_d, wqr_d, m) for m in range(2, HQL)] + \
                       [(kT, wk8_d, wkr_d, m) for m in range(HKVL)] + \
                       [(None, wv8_d, wvr_d, m) for m in range(HKVL)]
                w_cur = load_w(rest[0][1], rest[0][2], rest[0][3])
                for i, (dst, hi_d_, lo_d_, m) in enumerate(rest):
                    w_m = w_cur
                    if i + 1 < len(rest):
                        w_cur = load_w(rest[i + 1][1], rest[i + 1][2],
                                       rest[i + 1][3])
                    for c in range(NCH):
                        ps = psA.tile([128, 512], f32, tag="pj")
                        # hi-products first: the lo slab's DMA (prefetched
                        # or, for the first head, still in flight) hides
                        # under them.
                        for g in range(NG):
                            mm3_hi(ps, w_m, c, g, g == 0)
                        for g in range(NG):
                            mm3_lo(ps, w_m, c, g, g == NG - 1)
                        if dst is not None:
                            rope_evict(ps, dst[:, m, 512 * c:512 * (c + 1)],
                                       c)
                            continue
                        # V head: v^T psum -> sbuf -> PE transpose -> vON
                        vt = vtmp.tile([128, 512], b16, tag="vt")
                        nc.scalar.copy(out=vt[:], in_=ps[:])
                        pt = psT.tile([128, 512], b16, tag="tr")
                        for s in range(4):
                            nc.tensor.transpose(
                                pt[:, 128 * s:128 * (s + 1)],
                                vt[:, 128 * s:128 * (s + 1)], id_sb[:])
                        for s in range(4):
                            if i == len(rest) - 1:
                                # last head: DVE, so the ACT tail (exp
                                # chain) doesn't gate the phase barrier
                                nc.vector.tensor_copy(
                                    vON[:, m, 4 * c + s, 0:128],
                                    pt[:, 128 * s:128 * (s + 1)])
                            else:
                                nc.scalar.copy(
                                    out=vON[:, m, 4 * c + s, 0:128],
                                    in_=pt[:, 128 * s:128 * (s + 1)])
                        if i == len(rest) - 1:
                            # prewarm P^T(c=0, h=0) tile t=c: score + exp
                            # (+ causal mask) run here, under the last V
                            # head's matmuls, so phase B starts straight
                            # into h0's PV accumulation.
                            t_, col0 = c, 128 * c
                            pw = psA.tile([128, 512], f32, tag="pj",
                                          name=f"pw{t_}")
                            nc.tensor.matmul(
                                pw[:, col0:512],
                                lhsT=kT[:, 0, 128 * t_:128 * (t_ + 1)],
                                rhs=qT[:, 0, col0:512],
                                start=True, stop=True)
                            po = P0_OFF[t_]
                            nc.scalar.activation(
                                p0[:, po:po + 512 - col0],
                                pw[:, col0:512], Exp, scale=scale)
                            nc.gpsimd.tensor_tensor(
                                p0[:, po:po + 128],
                                p0[:, po:po + 128], tri_sb[:], mult)

            # ---------------- Phases B + C ---------------------------------
            with tc.tile_pool(name="late", bufs=1) as late, \
                 tc.tile_pool(name="ppool", bufs=20) as ppool, \
                 tc.tile_pool(name="npool", bufs=8) as npool, \
                 tc.tile_pool(name="spool", bufs=4) as spool, \
                 tc.tile_pool(name="psacc", bufs=1, space="PSUM") as psacc, \
                 tc.tile_pool(name="psP", bufs=2, space="PSUM") as psP, \
                 tc.tile_pool(name="psS", bufs=4, space="PSUM") as psS:

                # attention out per head, fp8 hi/lo planes (transposed)
                aS = late.tile([128, HQL, 2, T], f8)
                wp8_sb = late.tile([128, HQL, EOUT], f8)
                wpr_sb = late.tile([128, HQL, EOUT], f8)
                for k in range(HQL):
                    nc.sync.dma_start(out=wp8_sb[:, k, :], in_=wp8_d[:, k, :])
                    nc.sync.dma_start(out=wpr_sb[:, k, :], in_=wpr_d[:, k, :])

                # Phases B+C software-pipelined: while attention runs for
                # chunk c, the output projection for chunk c-1 is interleaved
                # between heads (4 e-tiles per head) so PE fills ACT-wait
                # gaps and the output DMA spreads across the whole run.
                def proj_tile(e, c, pool=None, tag="p"):
                    ps = (pool or psP).tile([128, 512], f32, tag=tag,
                                            name="psp")
                    NHG = HQL // 2
                    cs = slice(512 * c, 512 * (c + 1))
                    es = slice(128 * e, 128 * (e + 1))
                    for g in range(NHG):
                        h0 = 2 * g
                        nc.tensor.matmul(
                            ps[:],
                            lhsT=wp8_sb[:, h0, es].unsqueeze(1)
                                .broadcast_to([128, 2, 128]),
                            rhs=aS[:, h0, :, cs],
                            start=(g == 0), stop=False, perf_mode=DR)
                        nc.tensor.matmul(
                            ps[:],
                            lhsT=wpr_sb[:, h0:h0 + 2, es],
                            rhs=aS[:, h0:h0 + 2, 0, cs],
                            start=False, stop=False, perf_mode=DR)
                        nc.tensor.matmul(
                            ps[:],
                            lhsT=wp8_sb[:, h0 + 1, es].unsqueeze(1)
                                .broadcast_to([128, 2, 128]),
                            rhs=aS[:, h0 + 1, :, cs],
                            start=False, stop=(g == NHG - 1),
                            perf_mode=DR)
                    yt = ppool.tile([128, 512], b16, tag="yt", name="yt")
                    nc.scalar.copy(out=yt[:], in_=ps[:])
                    nc.sync.dma_start(
                        out=yt_d[128 * e:128 * (e + 1), 512 * c:512 * (c + 1)],
                        in_=yt[:])

                NE = EOUT // 128
                EPH = NE // HQL  # proj e-tiles interleaved per head
                pending = []     # deferred transpose+evict of previous head
                for c in range(NCH):
                    for h in range(HQL):
                        v = h // REP
                        # emit the previous head's a^T transposes now: their
                        # DVE normalize chain finished long ago, so PE does
                        # them back-to-back with no dependency stall.
                        for fn in pending:
                            fn()
                        pending = []
                        # two accs packed per psum bank: the s%2==0 start
                        # zeroes the whole bank (zero region), so the odd
                        # acc accumulates with start=False into pre-zeroed
                        # bytes (group check skipped by construction).
                        accp = [psacc.tile([128, 2, 129], f32, tag=f"accp{i}",
                                           name=f"accp{i}")
                                for i in range(2)]
                        accs = [accp[s // 2][:, s % 2, :] for s in range(4)]
                        n_tk = 4 * c + 4
                        pTs = {}

                        def vmms(t):
                            j = t - 4 * c
                            for s in range(4):
                                if j > s:
                                    continue
                                nc.tensor.matmul(
                                    accs[s],
                                    lhsT=pTs[t][:, 128 * s:128 * (s + 1)],
                                    rhs=vON[:, v, t, :],
                                    start=(t == 0 and s % 2 == 0),
                                    stop=(t == 4 * c + s),
                                    skip_group_check=True)

                        # proj tiles of the previous chunk, interleaved into
                        # the t-loop (own psum bank) to fill ACT-paced gaps
                        pe_list = (list(range(EPH * h, EPH * (h + 1)))
                                   if c > 0 else [])
                        D = 4  # score->exp->V software-pipeline depth
                        if c == 0 and h == 0:
                            # scores/exp prewarmed at the end of phase A:
                            # shifted views into p0 so pTs[t][:, 128s:...]
                            # lands on the compacted tile (valid for
                            # s >= j = t, the only slices vmms reads).
                            for t in range(n_tk):
                                o = P0_OFF[t] - 128 * t
                                pTs[t] = p0[:, o:o + 512]
                            for t in range(n_tk):
                                vmms(t)
                            n_tk = 0  # skip the emission loop below
                        for t in range(n_tk):
                            j = t - 4 * c  # >= 0 on diagonal-group tiles
                            col0 = 128 * j if j > 0 else 0
                            ps = psS.tile([128, 512], f32, tag="s")
                            nc.tensor.matmul(
                                ps[:, col0:512],
                                lhsT=kT[:, v, 128 * t:128 * (t + 1)],
                                rhs=qT[:, h, 512 * c + col0:512 * (c + 1)],
                                start=True, stop=True)
                            pT = ppool.tile([128, 512], b16, tag="pT")
                            nc.scalar.activation(
                                pT[:, col0:512], ps[:, col0:512], Exp,
                                scale=scale)
                            if j >= 0:
                                # causal mask applied to P (not scores): a
                                # 0/1 multiply on the idle GPSIMD engine
                                # (SBUF-only there, so legal), hidden
                                # behind the D-deep vmms slack, keeping
                                # the score->exp chain free of DVE hops.
                                nc.gpsimd.tensor_tensor(
                                    pT[:, 128 * j:128 * (j + 1)],
                                    pT[:, 128 * j:128 * (j + 1)],
                                    tri_sb[:], mult)
                            pTs[t] = pT
                            if t >= D:
                                vmms(t - D)
                            if pe_list and \
                               (t + 1) * EPH // n_tk > t * EPH // n_tk:
                                proj_tile(pe_list.pop(0), c - 1)
                        for t in range(max(0, n_tk - D), n_tk):
                            vmms(t)
                        for e in pe_list:
                            proj_tile(e, c - 1)
                        ans = []
                        for s in range(4):
                            rec = spool.tile([128, 1], f32, tag="rec")
                            nc.vector.reciprocal(rec[:], accs[s][:, 128:129])
                            an = npool.tile([128, 128], b16, tag="an")
                            nc.vector.tensor_scalar_mul(
                                an[:], accs[s][:, 0:128], rec[:])
                            ans.append(an)

                        def make_tr(ans=ans, h=h, c=c):
                            def emit():
                                pt = psP.tile([128, 512], b16, tag="p",
                                              name="pt")
                                for s in range(4):
                                    nc.tensor.transpose(
                                        pt[:, 128 * s:128 * (s + 1)],
                                        ans[s][:], id_sb[:])
                                cs = slice(512 * c, 512 * (c + 1))
                                nc.vector.tensor_copy(aS[:, h, 0, cs], pt[:])
                                nc.vector.tensor_tensor(
                                    aS[:, h, 1, cs], pt[:], aS[:, h, 0, cs],
                                    sub)
                            return emit

                        pending = [make_tr()]

                for fn in pending:
                    fn()
                # drain: projection of the last chunk. Scores are done, so
                # alternate between the proj bank and the (now idle) score
                # pool to double-buffer the drain and keep PE back-to-back.
                for e in range(NE - 1):
                    if e % 2 == 0:
                        proj_tile(e, NCH - 1)
                    else:
                        proj_tile(e, NCH - 1, pool=psS, tag="s")

                # last e-tile in two half-width groups (own banks) so the
                # final eviction+DMA pipelines instead of trailing the span.
                def proj_half(e, c, hf, pool, tag):
                    ps = pool.tile([128, 256], f32, tag=tag, name="psph")
                    NHG = HQL // 2
                    cs = slice(512 * c + 256 * hf, 512 * c + 256 * (hf + 1))
                    es = slice(128 * e, 128 * (e + 1))
                    for g in range(NHG):
                        h0 = 2 * g
                        nc.tensor.matmul(
                            ps[:],
                            lhsT=wp8_sb[:, h0, es].unsqueeze(1)
                                .broadcast_to([128, 2, 128]),
                            rhs=aS[:, h0, :, cs],
                            start=(g == 0), stop=False, perf_mode=DR)
                        nc.tensor.matmul(
                            ps[:],
                            lhsT=wpr_sb[:, h0:h0 + 2, es],
                            rhs=aS[:, h0:h0 + 2, 0, cs],
                            start=False, stop=False, perf_mode=DR)
                        nc.tensor.matmul(
                            ps[:],
                            lhsT=wp8_sb[:, h0 + 1, es].unsqueeze(1)
                                .broadcast_to([128, 2, 128]),
                            rhs=aS[:, h0 + 1, :, cs],
                            start=False, stop=(g == NHG - 1),
                            perf_mode=DR)
                    yt = ppool.tile([128, 256], b16, tag="yth", name="yth")
                    if hf == 1:
                        nc.vector.tensor_copy(yt[:], ps[:])
                    else:
                        nc.scalar.copy(out=yt[:], in_=ps[:])
                    nc.sync.dma_start(
                        out=yt_d[128 * e:128 * (e + 1),
                                 512 * c + 256 * hf:512 * c + 256 * (hf + 1)],
                        in_=yt[:])

                proj_half(NE - 1, NCH - 1, 0, psP, "p")
                proj_half(NE - 1, NCH - 1, 1, psS, "s")

    nc.compile()
    return nc


def _rope_tables(T=T):
    j = np.arange(64, dtype=np.float64)
    inv_freq = 1.0 / (BASE_FREQ ** (2.0 * j / HD))
    t = np.arange(T, dtype=np.float64)
    fr = t[:, None] * inv_freq[None, :]          # [T, 64]
    cos = np.cos(fr) / WS                        # fold 1/WS weight scale
    sin = np.sin(fr) / WS
    cos_tbl = np.concatenate([cos, cos], axis=1).T    # [128, T]
    sin_tbl = sin.T                                   # [64, T]
    # pack sin to [128, T/2]: rows 0:64 = cols 0:T/2, rows 64:128 = rest
    sin_tbl = np.concatenate([sin_tbl[:, :T // 2], sin_tbl[:, T // 2:]],
                             axis=0)
    return cos_tbl.astype(bf16), sin_tbl.astype(bf16)


def _split8(a):
    """fp32 -> (hi, lo) e4m3 pair with hi + lo ~= a."""
    hi = a.astype(e4m3)
    lo = (a - hi.astype(np.float32)).astype(e4m3)
    return hi, lo


def _pack_w8(w):
    """[KE, M] f32 -> two fp8 [128, M//128, KE//128, 128] (hi, lo):
    w_l[p, m, a, j] = w[128a+p, 128m+j]."""
    KE, M = w.shape
    packed = np.ascontiguousarray(
        w.reshape(KE // 128, 128, M // 128, 128).transpose(1, 2, 0, 3))
    return _split8(packed)


def prep_core_inputs(x, wq, wk, wv, wproj):
    cos_tbl, rsin_tbl = _rope_tables()
    tri = np.where(np.arange(128)[None, :] >= np.arange(128)[:, None],
                   1.0, 0.0).astype(bf16)
    ident = np.eye(128, dtype=bf16)
    in_maps = []
    xs_by_b = {}
    for b in range(B):
        xt = np.ascontiguousarray(
            x[b].T.reshape(N_EMBD // 128, 128, T).transpose(1, 0, 2))
        x8, xr8 = _split8(xt)
        xs_by_b[b] = np.ascontiguousarray(
            np.stack([x8, xr8], axis=2))       # [128, KT, 2, T]
    for ci in range(N_CORES):
        b, g = divmod(ci, TPG)
        qcols = slice(g * HQL * HD, (g + 1) * HQL * HD)
        kvcols = slice(g * HKVL * HD, (g + 1) * HKVL * HD)
        wq8, wqr = _pack_w8(wq[:, qcols] * WS)
        wk8, wkr = _pack_w8(wk[:, kvcols] * WS)
        wv8, wvr = _pack_w8(wv[:, kvcols] * WS)
        wpp = np.ascontiguousarray(
            (wproj[qcols, :] * WS).reshape(HQL, 128, N_EMBD)
            .transpose(1, 0, 2))
        wp8, wpr = _split8(wpp)
        in_maps.append({
            "xs": xs_by_b[b],
            "wq8": wq8, "wqr": wqr,
            "wk8": wk8, "wkr": wkr,
            "wv8": wv8, "wvr": wvr,
            "wp8": wp8, "wpr": wpr,
            "cos": cos_tbl, "rsin": rsin_tbl, "tri": tri, "ident": ident,
        })
    return in_maps


_NC_CACHE = {}


def _get_nc():
    if "nc" not in _NC_CACHE:
        _NC_CACHE["nc"] = build_nc()
    return _NC_CACHE["nc"]


def _get_runner():
    """Cached sharded-jit executor over the 8 cores (no donation, so the
    compiled executable is reusable across calls)."""
    if "runner" in _NC_CACHE:
        return _NC_CACHE["runner"]
    import jax
    from jax.sharding import Mesh, PartitionSpec, NamedSharding
    from jax.experimental.shard_map import shard_map
    from concourse import mybir
    from concourse.bass2jax import (_bass_exec_p, install_neuronx_cc_hook,
                                    partition_id_tensor)

    nc = _get_nc()
    install_neuronx_cc_hook()
    pname = nc.partition_id_tensor.name if nc.partition_id_tensor else None
    in_names, out_names, out_avals, zero_shapes = [], [], [], []
    for alloc in nc.m.functions[0].allocations:
        if not isinstance(alloc, mybir.MemoryLocationSet):
            continue
        name = alloc.memorylocations[0].name
        if alloc.kind == "ExternalInput":
            if name != pname:
                in_names.append(name)
        elif alloc.kind == "ExternalOutput":
            out_names.append(name)
            shape = tuple(alloc.tensor_shape)
            dtype = mybir.dt.np(alloc.dtype)
            out_avals.append(jax.core.ShapedArray(shape, dtype))
            zero_shapes.append((shape, dtype))
    all_names = in_names + out_names + ([pname] if pname else [])

    def _body(*args):
        operands = list(args)
        if pname:
            operands.append(partition_id_tensor())
        return tuple(_bass_exec_p.bind(
            *operands, out_avals=tuple(out_avals), in_names=tuple(all_names),
            out_names=tuple(out_names), lowering_input_output_aliases=(),
            sim_require_finite=True, sim_require_nnan=True, nc=nc))

    devices = jax.devices()[:N_CORES]
    mesh = Mesh(np.asarray(devices), ("core",))
    nin = len(in_names) + len(out_names)
    sharded = jax.jit(
        shard_map(_body, mesh=mesh, in_specs=(PartitionSpec("core"),) * nin,
                  out_specs=(PartitionSpec("core"),) * len(out_names),
                  check_rep=False),
        keep_unused=True)
    sh = NamedSharding(mesh, PartitionSpec("core"))
    zeros = [jax.device_put(
        np.zeros((N_CORES * s[0], *s[1:]), dt), sh)
        for s, dt in zero_shapes]

    def run(in_maps):
        concat = [np.concatenate([m[n] for m in in_maps], axis=0)
                  for n in in_names]
        dev_in = [jax.device_put(a, sh) for a in concat]
        outs = sharded(*dev_in, *zeros)
        jax.block_until_ready(outs)
        return [
            {n: np.asarray(outs[i]).reshape(N_CORES, *out_avals[i].shape)[ci]
             for i, n in enumerate(out_names)}
            for ci in range(N_CORES)]

    _NC_CACHE["runner"] = run
    return run


def kernel(x, wq, wk, wv, wproj):
    in_maps = prep_core_inputs(np.asarray(x, dtype=np.float32),
                               np.asarray(wq, dtype=np.float32),
                               np.asarray(wk, dtype=np.float32),
                               np.asarray(wv, dtype=np.float32),
                               np.asarray(wproj, dtype=np.float32))
    results = _get_runner()(in_maps)
    y = np.empty((B, T, N_EMBD), dtype=np.float32)
    for b in range(B):
        acc = results[b * TPG]["yt"].astype(np.float32)
        for g in range(1, TPG):
            acc += results[b * TPG + g]["yt"].astype(np.float32)
        y[b] = acc.T * (1.0 / WS)
    return y


if __name__ == "__main__":
    rng = np.random.default_rng(0)
    x = rng.standard_normal((B, T, N_EMBD), dtype=np.float32)
    wq_ = (rng.standard_normal((N_EMBD, N_EMBD), dtype=np.float32) * 0.02)
    wk_ = (rng.standard_normal((N_EMBD, HKV * HD), dtype=np.float32) * 0.02)
    wv_ = (rng.standard_normal((N_EMBD, HKV * HD), dtype=np.float32) * 0.02)
    wp_ = (rng.standard_normal((N_EMBD, N_EMBD), dtype=np.float32) * 0.02)
    y = kernel(x, wq_, wk_, wv_, wp_)
    print("out", y.shape, y.dtype, float(np.abs(y).max()))
